# revision 8
# baseline (speedup 1.0000x reference)
"""Bidirectional DragnnLSTM kernel, TP2 variant: 2 cores per direction.

Pairs (0,1)=lr, (2,3)=rl; cores 4-7 mirror 0-3.  Hidden dim H=1536 is
column-sharded 2 ways (CPG=768 per core).  Weights and exchange payloads are
bf16 (PSUM accumulation stays fp32); rel tolerance is 2e-2 so bf16 matvec
inputs are safe.

Why TP2: the per-step h/c exchange is bottlenecked by GPSIMD SWDGE
descriptor generation (~1us/descriptor, ~10 descriptors per remote write,
payload-size independent).  TP4 needs 6 remote writes per step (58us on
Pool); TP2 needs 2 (one peer), ~20us, which hides under the PE stream and
the cross-engine chain.

Differences from the TP4 baseline:
  - gate rows are [1,768] and span two PSUM banks (half-chains per bank);
  - slab tiles are [128,6]; exchange buffer c_full/h_full is [128,12]
    (slot 0 = own slab, slot 1 = XOR-1 peer slab), triple-buffered so the
    DVE never waits on send completion in steady state;
  - single-parity PSUM banks with WAR waits (consumers run well ahead);
  - cell state kept in fp32 (cpriv) with a bf16 mirror for matvec+exchange.
"""

import sys
import numpy as np
import ml_dtypes

sys.path.insert(0, "/opt/trn_rl_repo")

SEQ = 384
D_IN = 768
H = 1536
FF = 768
NA = 128
G = 2            # cores per direction
CPG = H // G     # 768 columns per core
HB = CPG // 2    # 384: half-row (per PSUM bank)
KH = H // 128    # 12 contraction chunks
NS = CPG // 128  # 6 slab chunks (tile columns per slab)
KX = 7           # x-side chunks: ceil(769/128)
KF = 24          # ff contraction chunks (3072/128)
MF = FF // 128   # 6 ff output chunks

_CACHE = {}


def _build(seq, no_exch=False, min_mm=False):
    import concourse.bass as bass
    import concourse.bacc as bacc
    import concourse.mybir as mybir

    F32 = mybir.dt.float32
    BF16 = mybir.dt.bfloat16
    AF = mybir.ActivationFunctionType
    tchunks = [(i * 128, min(128, seq - i * 128)) for i in range(((seq + 127) // 128))]
    TC = len(tchunks)

    nc = bacc.Bacc(target_bir_lowering=False)

    # ---------------- DRAM I/O ----------------
    xT_d = nc.dram_tensor("xT", [128, KX, seq], BF16, kind="ExternalInput")
    xw_d = {g: nc.dram_tensor(f"xw_{g}", [128, KX, CPG], BF16, kind="ExternalInput")
            for g in "ico"}
    W_d = {m: nc.dram_tensor(f"W_{m}", [128, KH, CPG], BF16, kind="ExternalInput")
           for m in ("hi", "ci", "hc", "ho", "co")}
    ident_d = nc.dram_tensor("ident", [128, 128], BF16, kind="ExternalInput")
    onesrow_d = nc.dram_tensor("onesrow", [128, 128], BF16, kind="ExternalInput")
    onesf_d = nc.dram_tensor("onesf", [128, 1], F32, kind="ExternalInput")
    hb1_d = nc.dram_tensor("hb1", [128, NA], BF16, kind="ExternalInput")
    ffw_d = nc.dram_tensor("ffw", [128, KF, FF], BF16, kind="ExternalInput")
    ffb_d = nc.dram_tensor("ffb", [128, MF], F32, kind="ExternalInput")
    hw_d = nc.dram_tensor("hw", [128, MF, NA], BF16, kind="ExternalInput")

    out_d = nc.dram_tensor("logits", [seq, NA], F32, kind="ExternalOutput")

    h_hist_f = nc.dram_tensor("h_hist_f", [128, NS * seq], BF16)  # [p, (k, t)]
    h_hist_r = nc.dram_tensor("h_hist_r", [128, NS * seq], BF16)  # token-reversed
    ag_out_f = nc.dram_tensor("ag_out_f", [8 * 128, NS * seq], BF16, addr_space="Shared")
    ag_out_r = nc.dram_tensor("ag_out_r", [8 * 128, NS * seq], BF16, addr_space="Shared")

    # ---------------- SBUF ----------------
    overlay_base = nc.sbuf_base  # FF tensors overlay the recurrence weights
    xT_s = nc.alloc_sbuf_tensor("xT_s", [128, KX, seq], BF16)
    xw_s = {g: nc.alloc_sbuf_tensor(f"xw_{g}_s", [128, KX, CPG], BF16) for g in "ico"}
    W_s = {m: nc.alloc_sbuf_tensor(f"W_{m}_s", [128, KH, CPG], BF16)
           for m in ("hi", "ci", "hc", "ho", "co")}
    ident_s = nc.alloc_sbuf_tensor("ident_s", [128, 128], BF16)
    onesrow_s = nc.alloc_sbuf_tensor("onesrow_s", [128, 128], BF16)
    hb1_s = nc.alloc_sbuf_tensor("hb1_s", [128, NA], BF16)
    ffb_s = nc.alloc_sbuf_tensor("ffb_s", [128, MF], F32)

    xp_s = {g: nc.alloc_sbuf_tensor(f"xp_{g}_s", [128, TC, CPG], BF16)
            for g in "ico"}

    h_full = [nc.alloc_sbuf_tensor(f"h_full{p}", [128, 2 * NS], BF16) for p in range(3)]
    c_full = [nc.alloc_sbuf_tensor(f"c_full{p}", [128, 2 * NS], BF16) for p in range(3)]
    cpriv = [nc.alloc_sbuf_tensor(f"cpriv{p}", [128, NS], F32) for p in range(2)]
    th_tile = nc.alloc_sbuf_tensor("th_tile", [128, NS], F32)

    row = lambda name: nc.alloc_sbuf_tensor(name, [1, CPG], F32)
    it_r, wt_r, ot_r = row("it_r"), row("wt_r"), row("ot_r")
    onesf_s = nc.alloc_sbuf_tensor("onesf_s", [128, 1], F32)

    hist_f_s = nc.alloc_sbuf_tensor("hist_f_s", [128, NS, seq], BF16)
    hist_r_s = nc.alloc_sbuf_tensor("hist_r_s", [128, NS, seq], BF16)

    # FF stage tensors overlay the recurrence weight region (phases are
    # strictly sequential; loads gated on the recurrence finishing)
    off0 = (overlay_base + 31) & ~31
    ffw_s = nc.alloc_sbuf_tensor_at("ffw_s", [128, KF, FF], BF16, offset=off0)
    off0 += KF * FF * 2
    ag_s = nc.alloc_sbuf_tensor_at("ag_s", [128, KF, seq], BF16, offset=off0)
    off0 += KF * seq * 2
    hidT_s = nc.alloc_sbuf_tensor_at("hidT_s", [128, MF, seq], BF16, offset=off0)
    hw_s = nc.alloc_sbuf_tensor("hw_s", [128, MF, NA], BF16)
    log_s = nc.alloc_sbuf_tensor("log_s", [128, TC, NA], F32)

    pb = [nc.place_psum_tensor(f"pb{i}", [128, 448], F32, bank=i) for i in range(8)]

    sems = {}
    semnames = ["LD", "XPMM", "XPCP",
                "GI", "GW", "GO",          # gate PSUM rows done (PE)
                "ACI", "ACW", "ACO", "ATH",  # activations done (Act)
                "TPIW", "TPO",             # transposes done (PE)
                "VCT", "VHT", "HCP",       # DVE tile writes / hist copies
                "RCT", "RHT", "BLS",       # exchange arrival / local completion
                "HSD", "CC", "LD2",        # epilogue
                "FFMM", "RELU", "HMM", "HCPY", "OUTD"]

    import contextlib
    ctx = contextlib.ExitStack()
    for s in semnames:
        sems[s] = ctx.enter_context(nc.semaphore(s))
    S = type("S", (), sems)

    load_list = [
        (xT_s.ap(), xT_d.ap()),
        (xw_s["i"].ap(), xw_d["i"].ap()), (xw_s["c"].ap(), xw_d["c"].ap()),
        (xw_s["o"].ap(), xw_d["o"].ap()),
        (W_s["hi"].ap(), W_d["hi"].ap()), (W_s["ci"].ap(), W_d["ci"].ap()),
        (W_s["hc"].ap(), W_d["hc"].ap()), (W_s["ho"].ap(), W_d["ho"].ap()),
        (W_s["co"].ap(), W_d["co"].ap()),
        (ident_s[:, :], ident_d[:, :]), (onesrow_s[:, :], onesrow_d[:, :]),
        (onesf_s[:, :], onesf_d[:, :]),
        (hb1_s[:, :], hb1_d[:, :]), (ffb_s[:, :], ffb_d[:, :]),
        (hw_s.ap(), hw_d.ap()),
    ]
    NLOADS = len(load_list)

    # xproj chains: (gate, tchunk_idx, half)
    xp_chains = [(g, i, hf) for i in range(TC) for g in "ico" for hf in (0, 1)]
    NXP = len(xp_chains)

    block_ctx = nc.Block()
    block = block_ctx.__enter__()

    # =================== SYNC: loads + epilogue stores ===================
    @block.sync
    def _(sync):
        for dst, src in load_list:
            sync.dma_start(out=dst, in_=src).then_inc(S.LD, 16)

        sync.wait_ge(S.HCP, seq)
        sync.dma_start(out=h_hist_f.ap(), in_=hist_f_s.ap()).then_inc(S.HSD, 16)
        sync.dma_start(out=h_hist_r.ap(), in_=hist_r_s.ap()).then_inc(S.HSD, 16)

        sync.wait_ge(S.GO, seq)
        sync.dma_start(out=ffw_s.ap(), in_=ffw_d.ap()).then_inc(S.LD2, 16)
        sync.wait_ge(S.CC, 2)
        half = KF // 2
        # lr halves (cores 0,1 of the quad group) from the forward gather,
        # rl halves (cores 2,3) from the reversed one
        sync.dma_start(
            out=ag_s[:, 0:half, :],
            in_=bass.AP(ag_out_f, 0,
                        [[NS * seq, 128], [128 * NS * seq, 2], [seq, NS], [1, seq]]),
        ).then_inc(S.LD2, 16)
        sync.dma_start(
            out=ag_s[:, half:KF, :],
            in_=bass.AP(ag_out_r, 2 * 128 * NS * seq,
                        [[NS * seq, 128], [128 * NS * seq, 2], [seq, NS], [1, seq]]),
        ).then_inc(S.LD2, 16)
        for i, (t0, tl) in enumerate(tchunks):
            sync.wait_ge(S.HCPY, i + 1)
            sync.dma_start(
                out=bass.AP(out_d, t0 * NA, [[NA, tl], [1, NA]]),
                in_=log_s[0:tl, i, :],
            ).then_inc(S.OUTD, 16)
        sync.wait_ge(S.OUTD, 16 * TC)

    # =================== TENSOR ===================
    @block.tensor
    def _(tensor):
        tensor.wait_ge(S.LD, 16 * NLOADS)

        # ---- x projections (half-rows [tl, 384] per bank) ----
        for idx, (g, i, hf) in enumerate(xp_chains):
            b = idx % 8
            if idx >= 8:
                tensor.wait_ge(S.XPCP, idx - 7)
            t0, tl = tchunks[i]
            c0 = hf * HB
            for k in range(KX):
                mm = tensor.matmul(pb[b][0:tl, 0:HB],
                                   xT_s[:, k, t0:t0 + tl],
                                   xw_s[g][:, k, c0:c0 + HB],
                                   start=(k == 0), stop=(k == KX - 1),
                                   skip_group_check=True)
            mm.then_inc(S.XPMM, 1)

        tensor.wait_ge(S.XPCP, NXP)

        # ---- recurrence ----
        # banks: i -> 0,1  w -> 2,3  o -> 4,5  transposes -> 6
        for t in range(seq):
            par3 = t % 3
            prev3 = (t - 1) % 3
            tci = t // 128
            trow = t % 128
            tlc = tchunks[tci][1]

            # WAR: sigmoid of step t-1 consumed banks 0/1.
            # c arrives mid-step t-1, h at its end: run the c2i chains before
            # waiting on the h arrival so the PE streams during the h flight.
            if t >= 1:
                tensor.wait_ge(S.ACI, t)
                tensor.wait_ge(S.VCT, t)
                if not no_exch:
                    tensor.wait_ge(S.RCT, 2 * t)
                for hf in (0, 1):
                    B = pb[hf]
                    c0 = hf * HB
                    for k in range(KH if not min_mm else 1):
                        tensor.matmul(B[0:1, 0:HB], c_full[prev3][:, k:k + 1],
                                      W_s["ci"][:, k, c0:c0 + HB],
                                      start=(k == 0), stop=False,
                                      tile_position=(0, 0), skip_group_check=True)
                tensor.wait_ge(S.VHT, t)
                if not no_exch:
                    tensor.wait_ge(S.RHT, 2 * t)
            for hf in (0, 1):
                B = pb[hf]
                c0 = hf * HB
                if t >= 1:
                    for k in range(KH if not min_mm else 1):
                        tensor.matmul(B[0:1, 0:HB], h_full[prev3][:, k:k + 1],
                                      W_s["hi"][:, k, c0:c0 + HB],
                                      start=False, stop=False,
                                      tile_position=(0, 0), skip_group_check=True)
                mm = tensor.matmul(B[0:1, 0:HB], ident_s[0:tlc, trow:trow + 1],
                                   xp_s["i"][0:tlc, tci, c0:c0 + HB],
                                   start=(t == 0), stop=True,
                                   tile_position=(0, 0), skip_group_check=True)
            mm.then_inc(S.GI, 1)

            # w chain: banks 2,3
            if t >= 1:
                tensor.wait_ge(S.ACW, t)
            for hf in (0, 1):
                B = pb[2 + hf]
                c0 = hf * HB
                if t >= 1:
                    for k in range(KH if not min_mm else 1):
                        tensor.matmul(B[0:1, 0:HB], h_full[prev3][:, k:k + 1],
                                      W_s["hc"][:, k, c0:c0 + HB],
                                      start=(k == 0), stop=False,
                                      tile_position=(0, 0), skip_group_check=True)
                mm = tensor.matmul(B[0:1, 0:HB], ident_s[0:tlc, trow:trow + 1],
                                   xp_s["c"][0:tlc, tci, c0:c0 + HB],
                                   start=(t == 0), stop=True,
                                   tile_position=(0, 0), skip_group_check=True)
            mm.then_inc(S.GW, 1)

            # transposes into bank 6: i cols 0..5, w cols 6..11, o cols 12..17
            # WAR: DVE h-update of step t-1 consumed o-cols
            if t >= 1:
                tensor.wait_ge(S.VHT, t)
            tensor.wait_ge(S.ACI, t + 1)
            for k in range(NS):
                tensor.matmul(pb[6][0:128, k:k + 1],
                              it_r[0:1, 128 * k:128 * (k + 1)],
                              onesf_s[0:1, 0:1],
                              is_transpose=True, skip_group_check=True)
            tensor.wait_ge(S.ACW, t + 1)
            for k in range(NS):
                mm = tensor.matmul(pb[6][0:128, NS + k:NS + k + 1],
                                   wt_r[0:1, 128 * k:128 * (k + 1)],
                                   onesf_s[0:1, 0:1],
                                   is_transpose=True, skip_group_check=True)
            mm.then_inc(S.TPIW, 1)

            # o chain part 1: banks 4,5 (groups stay open); placed here so the
            # h2o stream overlaps the DVE c-update and the c-exchange flight
            if t >= 1:
                tensor.wait_ge(S.ACO, t)
            for hf in (0, 1):
                B = pb[4 + hf]
                c0 = hf * HB
                if t >= 1:
                    for k in range(KH if not min_mm else 1):
                        tensor.matmul(B[0:1, 0:HB], h_full[prev3][:, k:k + 1],
                                      W_s["ho"][:, k, c0:c0 + HB],
                                      start=(k == 0), stop=False,
                                      tile_position=(0, 0), skip_group_check=True)
                tensor.matmul(B[0:1, 0:HB], ident_s[0:tlc, trow:trow + 1],
                              xp_s["o"][0:tlc, tci, c0:c0 + HB],
                              start=(t == 0), stop=False,
                              tile_position=(0, 0), skip_group_check=True)

            # o chain part 2: c2o on fresh ct
            tensor.wait_ge(S.VCT, t + 1)
            if not no_exch:
                tensor.wait_ge(S.RCT, 2 * (t + 1))
            for hf in (0, 1):
                B = pb[4 + hf]
                c0 = hf * HB
                KHe = KH if not min_mm else 1
                for k in range(KHe):
                    mm = tensor.matmul(B[0:1, 0:HB], c_full[par3][:, k:k + 1],
                                       W_s["co"][:, k, c0:c0 + HB],
                                       start=False, stop=(k == KHe - 1),
                                       tile_position=(0, 0), skip_group_check=True)
            mm.then_inc(S.GO, 1)

            # transpose of activated o row
            tensor.wait_ge(S.ACO, t + 1)
            for k in range(NS):
                mm = tensor.matmul(pb[6][0:128, 2 * NS + k:2 * NS + k + 1],
                                   ot_r[0:1, 128 * k:128 * (k + 1)],
                                   onesf_s[0:1, 0:1],
                                   is_transpose=True, skip_group_check=True)
            mm.then_inc(S.TPO, 1)

        # ---- FF ----
        tensor.wait_ge(S.LD2, 48)
        for m in range(MF):
            for k in range(KF):
                mm = tensor.matmul(pb[m][0:128, 0:seq],
                                   ffw_s[:, k, 128 * m:128 * (m + 1)],
                                   ag_s[:, k, :],
                                   start=(k == 0), stop=(k == KF - 1),
                                   skip_group_check=True)
            mm.then_inc(S.FFMM, 1)
        tensor.wait_ge(S.RELU, MF)
        for i, (t0, tl) in enumerate(tchunks):
            b = pb[6 + (i % 2)]
            if i >= 2:
                tensor.wait_ge(S.HCPY, i - 1)
            for k in range(MF):
                tensor.matmul(b[0:tl, 0:NA], hidT_s[:, k, t0:t0 + tl],
                              hw_s[:, k, :],
                              start=(k == 0), stop=False, skip_group_check=True)
            tensor.matmul(b[0:tl, 0:NA], onesrow_s[:, 0:tl], hb1_s[:, :],
                          start=False, stop=True, skip_group_check=True
                          ).then_inc(S.HMM, 1)

    # =================== VECTOR (DVE) ===================
    @block.vector
    def _(vector):
        # xproj copies (halves)
        for idx, (g, i, hf) in enumerate(xp_chains):
            b = idx % 8
            vector.wait_ge(S.XPMM, idx + 1)
            t0, tl = tchunks[i]
            c0 = hf * HB
            vector.tensor_copy(xp_s[g][0:tl, i, c0:c0 + HB],
                               pb[b][0:tl, 0:HB]).then_inc(S.XPCP, 1)

        for t in range(seq):
            par3 = t % 3
            par2 = t % 2
            prev2 = (t - 1) % 2
            i_cols = pb[6][0:128, 0:NS]
            w_cols = pb[6][0:128, NS:2 * NS]
            o_cols = pb[6][0:128, 2 * NS:3 * NS]
            cpar = cpriv[par2]
            cprev = cpriv[prev2]

            vector.wait_ge(S.TPIW, t + 1)
            if t >= 3 and not no_exch:
                # triple-buffered exchange tiles: sends of step t-3 complete
                vector.wait_ge(S.BLS, 32 * (t - 2))
            if t >= 2:
                # WAR on cpriv[par2]: ACT tanh of step t-2 consumed it
                vector.wait_ge(S.ATH, t - 1)
            if t == 0:
                vector.tensor_copy(cpar[:, :], w_cols)
                vector.drain()
                vector.tensor_mul(cpar[:, :], i_cols, cpar[:, :])
            else:
                vector.tensor_sub(cpar[:, :], w_cols, cprev[:, :])
                vector.drain()
                vector.tensor_mul(cpar[:, :], i_cols, cpar[:, :])
                vector.drain()
                vector.tensor_add(cpar[:, :], cpar[:, :], cprev[:, :])
            vector.drain()
            vector.tensor_copy(c_full[par3][:, 0:NS], cpar[:, :]).then_inc(S.VCT, 1)

            vector.wait_ge(S.TPO, t + 1)
            vector.wait_ge(S.ATH, t + 1)
            vector.tensor_mul(h_full[par3][:, 0:NS], o_cols, th_tile[:, :]
                              ).then_inc(S.VHT, 1)
            vector.drain()
            vector.tensor_copy(hist_f_s[:, :, t], h_full[par3][:, 0:NS])
            vector.tensor_copy(hist_r_s[:, :, seq - 1 - t], h_full[par3][:, 0:NS]
                               ).then_inc(S.HCP, 1)

        # head copies
        for i, (t0, tl) in enumerate(tchunks):
            b = pb[6 + (i % 2)]
            vector.wait_ge(S.HMM, i + 1)
            vector.tensor_copy(log_s[0:tl, i, :], b[0:tl, 0:NA]).then_inc(S.HCPY, 1)

    # =================== SCALAR (ACT) ===================
    @block.scalar
    def _(scalar):
        for t in range(seq):
            par2 = t % 2
            scalar.wait_ge(S.GI, t + 1)
            scalar.activation(it_r[0:1, 0:HB], pb[0][0:1, 0:HB], AF.Sigmoid)
            scalar.activation(it_r[0:1, HB:CPG], pb[1][0:1, 0:HB], AF.Sigmoid
                              ).then_inc(S.ACI, 1)
            scalar.wait_ge(S.GW, t + 1)
            scalar.activation(wt_r[0:1, 0:HB], pb[2][0:1, 0:HB], AF.Tanh)
            scalar.activation(wt_r[0:1, HB:CPG], pb[3][0:1, 0:HB], AF.Tanh
                              ).then_inc(S.ACW, 1)
            scalar.wait_ge(S.VCT, t + 1)
            scalar.activation(th_tile[:, :], cpriv[par2][:, :], AF.Tanh
                              ).then_inc(S.ATH, 1)
            scalar.wait_ge(S.GO, t + 1)
            scalar.activation(ot_r[0:1, 0:HB], pb[4][0:1, 0:HB], AF.Sigmoid)
            scalar.activation(ot_r[0:1, HB:CPG], pb[5][0:1, 0:HB], AF.Sigmoid
                              ).then_inc(S.ACO, 1)

        # FF relu with per-partition bias
        for m in range(MF):
            scalar.wait_ge(S.FFMM, m + 1)
            scalar.activation(hidT_s[:, m, :],
                              pb[m][0:128, 0:seq], AF.Relu,
                              bias=ffb_s[:, m:m + 1]).then_inc(S.RELU, 1)

    # =================== GPSIMD: exchange ======
    @block.gpsimd
    def _(gpsimd):
        def gen_bcast(buf, sem):
            rd = [None] * 8
            rd[1] = (0, 1)
            gpsimd.remote_dma_broadcast(
                out_ap=buf[:, NS:2 * NS],
                in_ap=buf[:, 0:NS],
                remote_sem=sem, local_sem=S.BLS,
                rdests=rd,
            )

        gpsimd.wait_ge(S.LD, 16 * NLOADS)
        if not no_exch:
            # FIFO order must match trigger order: c0, h0, c1, h1, ...
            gen_bcast(c_full[0], S.RCT)          # c0
            gen_bcast(h_full[0], S.RHT)          # h0
            gen_bcast(c_full[1], S.RCT)          # c1
            for t in range(seq):
                gpsimd.wait_ge(S.VCT, t + 1)
                gpsimd.trigger_dma(1)            # fire c(t)
                if t + 1 < seq:
                    gen_bcast(h_full[(t + 1) % 3], S.RHT)
                gpsimd.wait_ge(S.VHT, t + 1)
                gpsimd.trigger_dma(1)            # fire h(t)
                if t + 2 < seq:
                    gen_bcast(c_full[(t + 2) % 3], S.RCT)

        # collectives once the hist stores are done
        gpsimd.wait_ge(S.HSD, 32)
        import concourse.mybir as mybir2
        gpsimd.collective_compute(
            "AllGather",
            mybir2.AluOpType.bypass,
            replica_groups=[list(range(8))],
            ins=[h_hist_f.ap().opt()],
            outs=[ag_out_f.ap().opt()],
        ).then_inc(S.CC)
        gpsimd.collective_compute(
            "AllGather",
            mybir2.AluOpType.bypass,
            replica_groups=[list(range(8))],
            ins=[h_hist_r.ap().opt()],
            outs=[ag_out_r.ap().opt()],
        ).then_inc(S.CC)

    block_ctx.__exit__(None, None, None)
    ctx.close()
    nc.compile()
    return nc


# ---------------- host-side helpers ----------------

def _perm_index(g):
    # E[p, k] = global h index stored at tile position [p, k] on a core with
    # group index g: slot s = k//NS holds the slab of group g ^ s; within the
    # slab, position (p, kk=k%NS) holds element 128*kk + p.
    p = np.arange(128)[:, None]
    k = np.arange(KH)[None, :]
    return CPG * ((g ^ (k // NS)) & 1) + 128 * (k % NS) + p


def _bf(a):
    return np.asarray(a, np.float32).astype(ml_dtypes.bfloat16)


def _prep_core_inputs(c, inp):
    cc = c % 4
    pre = "lr" if cc < G else "rl"
    g = cc % G
    Sl = slice(CPG * g, CPG * (g + 1))
    E = _perm_index(g)

    x = np.asarray(inp["x"], np.float32)
    xin = x if pre == "lr" else x[::-1]
    seq = x.shape[0]

    x_aug = np.zeros((KX * 128, seq), np.float32)
    x_aug[:D_IN] = xin.T
    x_aug[D_IN] = 1.0
    xT = np.ascontiguousarray(x_aug.reshape(KX, 128, seq).transpose(1, 0, 2))

    def aug_w(w, b):
        wa = np.zeros((KX * 128, CPG), np.float32)
        wa[:D_IN] = w[:, Sl]
        wa[D_IN] = b[Sl]
        return np.ascontiguousarray(wa.reshape(KX, 128, CPG).transpose(1, 0, 2))

    def perm_w(w):
        return np.ascontiguousarray(np.asarray(w, np.float32)[:, Sl][E])

    # ffw rows: gather chunk kf: kf < 12 -> lr, kf >= 12 -> rl.
    # Within a half: sub-core q = kf//NS (0..1), kk = kf%NS:
    # global ff_in row = dir*1536 + q*768 + 128*kk + p.
    ffw = np.asarray(inp["ff_w"], np.float32)
    ffw_prep = np.empty((128, KF, FF), np.float32)
    for kf in range(KF):
        d, rem = kf // (KF // 2), kf % (KF // 2)
        q, kk = rem // NS, rem % NS
        base = d * H + q * CPG + 128 * kk
        ffw_prep[:, kf, :] = ffw[base:base + 128, :]

    d = {
        "xT": _bf(xT),
        "xw_i": _bf(aug_w(inp[pre + "_x2i"], inp[pre + "_bi"])),
        "xw_c": _bf(aug_w(inp[pre + "_x2c"], inp[pre + "_bc"])),
        "xw_o": _bf(aug_w(inp[pre + "_x2o"], inp[pre + "_bo"])),
        "W_hi": _bf(perm_w(inp[pre + "_h2i"])),
        "W_ci": _bf(perm_w(inp[pre + "_c2i"])),
        "W_hc": _bf(perm_w(inp[pre + "_h2c"])),
        "W_ho": _bf(perm_w(inp[pre + "_h2o"])),
        "W_co": _bf(perm_w(inp[pre + "_c2o"])),
        "ident": _bf(np.eye(128, dtype=np.float32)),
        "onesrow": np.zeros((128, 128), ml_dtypes.bfloat16),
        "onesf": np.zeros((128, 1), np.float32),
        "hb1": np.zeros((128, NA), ml_dtypes.bfloat16),
        "ffw": _bf(ffw_prep),
        "ffb": np.ascontiguousarray(
            np.asarray(inp["ff_b"], np.float32).reshape(MF, 128).T),
        "hw": _bf(np.asarray(inp["head_w"], np.float32).reshape(MF, 128, NA)
                  .transpose(1, 0, 2)),
    }
    d["onesrow"][0, :] = np.float32(1.0)
    d["onesf"][0, 0] = 1.0
    d["hb1"][0, :] = _bf(np.asarray(inp["head_b"], np.float32))
    return d


def kernel(**inputs):
    from concourse.bass_utils import run_bass_kernel_spmd

    seq = np.asarray(inputs["x"]).shape[0]
    if "nc" not in _CACHE or _CACHE.get("seq") != seq:
        _CACHE["nc"] = _build(seq)
        _CACHE["seq"] = seq
    nc = _CACHE["nc"]

    in_maps = [_prep_core_inputs(c, inputs) for c in range(8)]
    res = run_bass_kernel_spmd(nc, in_maps, core_ids=list(range(8)), trace=False)
    return res.results[0]["logits"]
